# revision 24
# baseline (speedup 1.0000x reference)
"""GCN (2x GCNConv + graclus-style max-pool head) on 8 Trainium2 NeuronCores.

Strategy (graph partitioning per the sharding hint):
  - Nodes are sharded contiguously across 8 cores (12500 each, padded to
    12544 = 98 tiles of 128).  Edges are partitioned by destination node.
  - deg / dinv = 1/sqrt(deg) are computed fully locally (all edges with a
    given dst live on its owner core).
  - Per layer: each core computes dinv * (x_shard @ W) locally, then an
    8-rank AllGather replicates the full [100352, 64] feature table.
  - Edge pass: per 128-edge chunk, dma_gather pulls table[src] rows into
    SBUF (4 src shards -> 4 SWDGE queues so transfers overlap ~4x), the
    Scalar engine converts them to bf16 while permuting from shard-major
    to tile-major chunk order, a bf16 selection block
    sel[e, c, n] = (iota[n] == dst_local[e, c]) * w[e, c] is built for all
    of a tile's chunks with two broadcast tensor_tensor ops, and the
    TensorEngine accumulates sel.T @ gathered into the per-tile PSUM
    (segment-sum).
  - Self loops are NOT gathered: their contribution is exactly the local
    table tile, added in the per-tile epilogue before the dinv scaling.
  - Pooling head: out[b] = max(h2[2c], h2[2c+1]) for the first cluster c of
    graph b; those 512 rows are fetched with one small dma_gather and
    reduced with a single elementwise max.
"""

import os
import sys

sys.path.insert(0, "/opt/trn_rl_repo")

import numpy as np
import ml_dtypes

N = 100000
E = 1600000
B = 256
IN_DIM = 128
OUT_DIM = 64
NCORES = 8
NS = N // NCORES          # 12500 real nodes per core
NT = (NS + 127) // 128    # 98 tiles per core
NSP = NT * 128            # 12544 padded nodes per core
GT = 7                    # tiles per gather group
NG = NT // GT             # 14 groups
NSH = 4                   # src table shards (int16 gather index limit)
# four piece-tables, one per SWDGE queue; piece p covers padded-local rows
# [PB[p], PB[p+1]) of every core, gathered by its own incremental AllGather
PB = [0, 3072, 6272, 9472, 12544]
PROWS = [3072, 3200, 3200, 3072]
PTILES = [24, 25, 25, 24]
P = 128
D = OUT_DIM


def _prepare(inputs):
    x = np.asarray(inputs["x"], dtype=np.float32)
    edge_index = np.asarray(inputs["edge_index"], dtype=np.int64)
    edge_weight = np.asarray(inputs["edge_weight"], dtype=np.float32)
    batch = np.asarray(inputs["batch"], dtype=np.int64)
    W1 = np.asarray(inputs["W1"], dtype=np.float32)
    b1 = np.asarray(inputs["b1"], dtype=np.float32)
    W2 = np.asarray(inputs["W2"], dtype=np.float32)
    b2 = np.asarray(inputs["b2"], dtype=np.float32)

    # real edges only; self loops are handled via an identity matmul on the
    # local table tile in the PSUM chain
    src = edge_index[0]
    dst = edge_index[1]
    w = edge_weight

    core = dst // NS
    lt = dst - core * NS            # local node id 0..12499
    t = lt // P                     # tile 0..97
    dl = (lt - t * P).astype(np.float32)
    lsrc = src % NS
    pb = np.asarray(PB, np.int64)
    prows = np.asarray(PROWS, np.int64)
    s = np.searchsorted(pb, lsrc, side="right") - 1   # piece = shard 0..3
    rowid = (src // NS) * prows[s] + (lsrc - pb[s])
    li = rowid.astype(np.int16)

    # per (core, tile, shard) counts -> global chunk capacities K[t, s]
    key = ((core * NT + t) * NSH + s).astype(np.int64)
    cnt = np.bincount(key, minlength=NCORES * NT * NSH).reshape(NCORES, NT, NSH)
    K = ((cnt.max(axis=0) + P - 1) // P).astype(np.int64)   # [NT, NSH]

    # shard-major layout (gather output): for group g: for s: for t: K chunks
    cb0 = np.zeros((NG, NSH), np.int64)       # group-local chunk base of (g, s)
    toff = np.zeros((NT, NSH), np.int64)      # chunk offset of tile within (g, s)
    NIgs = np.zeros((NG, NSH), np.int64)      # idxs per gather instruction
    Cg = np.zeros(NG, np.int64)               # chunks per group
    for g in range(NG):
        tl = range(g * GT, (g + 1) * GT)
        off = 0
        for sh in range(NSH):
            cb0[g, sh] = off
            o2 = 0
            for tt in tl:
                toff[tt, sh] = o2
                o2 += K[tt, sh]
            NIgs[g, sh] = P * o2
            off += o2
        Cg[g] = off
    gbase = np.concatenate([[0], np.cumsum(Cg)])   # group chunk base, [NG+1]
    TC = int(gbase[-1])                            # total chunks per layer
    icb = np.zeros((NG, NSH), np.int64)            # idx col base per instruction
    run = 0
    for g in range(NG):
        for sh in range(NSH):
            icb[g, sh] = run
            run += NIgs[g, sh] // 16
    ICOLS = int(run)

    gidx_of_t = np.arange(NT) // GT
    # shard-major slot base of each (t, s) cell (gather rows land here)
    cellbase_sm = P * (gbase[gidx_of_t][:, None] + cb0[gidx_of_t, :] + toff)

    # tile-major layout (bf16 buffer / sel / matmul): for t: for s: K chunks
    tb_of_t = np.zeros(NT, np.int64)          # chunk base of tile within group
    soff = np.zeros((NT, NSH), np.int64)      # chunk offset of shard within tile
    ntc = np.zeros(NT, np.int64)              # chunks per tile
    for g in range(NG):
        off = 0
        for tt in range(g * GT, (g + 1) * GT):
            tb_of_t[tt] = off
            o2 = 0
            for sh in range(NSH):
                soff[tt, sh] = o2
                o2 += K[tt, sh]
            ntc[tt] = o2
            off += o2
    MT = int(ntc.max())
    cellbase_tm = P * (gbase[gidx_of_t][:, None] + tb_of_t[:, None] + soff)

    # in-degree slot layout for deg computation (original edges only)
    dst0 = edge_index[1]
    corei = dst0 // NS
    lni = dst0 - corei * NS
    keyd = corei * NS + lni
    cntd = np.bincount(keyd, minlength=NCORES * NS).reshape(NCORES, NS)
    CAPD = int(cntd.max())

    # pooling: first cluster per graph (exact reference semantics)
    ncl = N // 2
    bp = batch[0::2]
    first = np.full(B, np.iinfo(np.int32).max, np.int64)
    np.minimum.at(first, bp, np.arange(ncl, dtype=np.int64))
    cl = np.clip(first, 0, ncl - 1)
    row_even = 2 * cl
    owner = (row_even // NS).astype(np.int64)
    loc_even = row_even - owner * NS

    iota_np = np.broadcast_to(np.arange(P, dtype=np.float32),
                              (1, P)).astype(ml_dtypes.bfloat16)
    iota_np = np.broadcast_to(iota_np[None, :, :], (P, MT, P)).copy()
    b1r = np.broadcast_to(b1.astype(np.float32), (P, D)).copy()
    b2r = np.broadcast_to(b2.astype(np.float32), (P, D)).copy()

    in_maps = []
    for c in range(NCORES):
        m = core == c
        tt = t[m]
        ss = s[m]
        cell = tt * NSH + ss
        order = np.argsort(cell, kind="stable")
        cell_s = cell[order]
        cnt_c = np.bincount(cell, minlength=NT * NSH)
        starts = np.concatenate([[0], np.cumsum(cnt_c)])[:-1]
        rank = np.arange(cell_s.size) - starts[cell_s]
        slot_sm = cellbase_sm.reshape(-1)[cell_s] + rank
        slot_tm = cellbase_tm.reshape(-1)[cell_s] + rank

        # tile-major dst lanes (for the on-device one-hot) and shard-major
        # per-edge weights (folded into gathered rows per chunk)
        dstl_arr = np.full((P, TC), -1.0, np.float32)
        lane = slot_tm % P
        cpos = slot_tm // P
        dstl_arr[lane, cpos] = dl[m][order]
        w_arr = np.zeros((P, TC), np.float32)
        w_arr[slot_sm % P, slot_sm // P] = w[m][order]

        gidx_arr = np.zeros((P, ICOLS), np.int16)
        inst = gidx_of_t[tt[order]] * NSH + ss[order]
        instr_chunk_base = (gbase[gidx_of_t][:, None] + cb0[gidx_of_t, :])
        j = slot_sm - P * instr_chunk_base.reshape(NT, NSH)[tt[order], ss[order]]
        colb = icb.reshape(-1)[inst]
        col = colb + j // 16
        row16 = (j % 16).astype(np.int64)
        liv = li[m][order]
        for rep in range(8):
            gidx_arr[16 * rep + row16, col] = liv

        # degree slots
        md = corei == c
        lnc = lni[md]
        od = np.argsort(lnc, kind="stable")
        lns = lnc[od]
        startsd = np.concatenate([[0], np.cumsum(cntd[c])])[:-1]
        rankd = np.arange(lns.size) - startsd[lns]
        degw_arr = np.zeros((P, NT, CAPD), np.float32)
        degw_arr[lns % P, lns // P, rankd] = edge_weight[md][od]

        # pooling gather indices (512: evens then odds), 0 for non-owned
        pe = np.where(owner == c, loc_even, 0).astype(np.int64)
        po = np.where(owner == c, loc_even + 1, 0).astype(np.int64)
        pidx_flat = np.concatenate([pe, po]).astype(np.int16)
        pidx_arr = np.zeros((P, 32), np.int16)
        jj = np.arange(512)
        for rep in range(8):
            pidx_arr[16 * rep + jj % 16, jj // 16] = pidx_flat

        xT = np.zeros((P, NSP), ml_dtypes.bfloat16)
        xT[:, :NS] = x[c * NS:(c + 1) * NS].T.astype(ml_dtypes.bfloat16)

        in_maps.append({
            "xT": xT,
            "degw": degw_arr.reshape(P, NT * CAPD),
            "dstl": dstl_arr.astype(ml_dtypes.bfloat16),
            "wsm": w_arr.astype(ml_dtypes.bfloat16),
            "iota": iota_np,
            "identbf": np.eye(P, dtype=ml_dtypes.bfloat16),
            "gidx": gidx_arr,
            "pidx": pidx_arr,
            "W1": W1.astype(ml_dtypes.bfloat16),
            "W2": W2,
            "b1r": b1r,
            "b2r": b2r,
        })

    tables = dict(K=K, NIgs=NIgs, cb0=cb0, toff=toff, gbase=gbase, Cg=Cg,
                  icb=icb, TC=TC, ICOLS=ICOLS, CAPD=CAPD,
                  tb_of_t=tb_of_t, soff=soff, ntc=ntc, MT=MT,
                  has_b1=bool(np.any(b1 != 0)), has_b2=bool(np.any(b2 != 0)))
    return in_maps, tables, owner


def _build(tables):
    import concourse.bass as bass
    import concourse.tile as tile
    from concourse import mybir, bacc, library_config

    K = tables["K"]
    NIgs = tables["NIgs"]
    cb0 = tables["cb0"]
    toff = tables["toff"]
    gbase = tables["gbase"]
    Cg = tables["Cg"]
    icb = tables["icb"]
    TC = tables["TC"]
    ICOLS = tables["ICOLS"]
    CAPD = tables["CAPD"]
    tb_of_t = tables["tb_of_t"]
    soff = tables["soff"]
    ntc = tables["ntc"]
    MT = tables["MT"]
    has_b1 = tables["has_b1"]
    has_b2 = tables["has_b2"]

    f32 = mybir.dt.float32
    bf16 = mybir.dt.bfloat16
    i16 = mybir.dt.int16
    AOP = mybir.AluOpType

    nc = bacc.Bacc("TRN2", target_bir_lowering=False, debug=False,
                   num_devices=NCORES, dynamic_dma_scratch_size=20480,
                   num_swdge_queues=4)

    xT = nc.declare_dram_parameter("xT", [P, NSP], bf16, isOutput=False)
    degw = nc.declare_dram_parameter("degw", [P, NT * CAPD], f32, isOutput=False)
    dstl = nc.declare_dram_parameter("dstl", [P, TC], bf16, isOutput=False)
    wsm = nc.declare_dram_parameter("wsm", [P, TC], bf16, isOutput=False)
    iota = nc.declare_dram_parameter("iota", [P, MT, P], bf16, isOutput=False)
    identbf = nc.declare_dram_parameter("identbf", [P, P], bf16, isOutput=False)
    gidx = nc.declare_dram_parameter("gidx", [P, ICOLS], i16, isOutput=False)
    pidx = nc.declare_dram_parameter("pidx", [P, 32], i16, isOutput=False)
    W1 = nc.declare_dram_parameter("W1", [IN_DIM, D], bf16, isOutput=False)
    W2 = nc.declare_dram_parameter("W2", [D, D], f32, isOutput=False)
    b1r = nc.declare_dram_parameter("b1r", [P, D], f32, isOutput=False)
    b2r = nc.declare_dram_parameter("b2r", [P, D], f32, isOutput=False)
    pool_out = nc.declare_dram_parameter("pool_out", [P, 2, D], f32, isOutput=True)

    tab1_mine = nc.dram_tensor("tab1_mine", [NSP, P], bf16)
    tab2_mine = nc.dram_tensor("tab2_mine", [NSP, P], bf16)
    tab1_f = [nc.dram_tensor(f"tab1_f{p}", [NCORES * PROWS[p], P], bf16,
                             addr_space="Shared") for p in range(NSH)]
    tab2_f = [nc.dram_tensor(f"tab2_f{p}", [NCORES * PROWS[p], P], bf16,
                             addr_space="Shared") for p in range(NSH)]
    h2_local = nc.dram_tensor("h2_local", [NSP, D], f32)

    groups = [list(range(NCORES))]

    # per-tile shard-major chunk positions, enumerated (s, k) = tile-major
    tile_chunks = []
    for t in range(NT):
        g = t // GT
        lst = []
        for s in range(NSH):
            base = cb0[g, s] + toff[t, s]
            for k in range(int(K[t, s])):
                lst.append(int(base + k))
        tile_chunks.append(lst)

    from contextlib import ExitStack
    with ExitStack() as top:
        tc = top.enter_context(tile.TileContext(nc))
        nc.gpsimd.load_library(library_config.mlp)
        const = top.enter_context(tc.tile_pool(name="const", bufs=1))
        W1_t = const.tile([IN_DIM, D], bf16)
        nc.sync.dma_start(out=W1_t[:], in_=W1[:])
        W2_t = const.tile([D, D], f32)
        nc.sync.dma_start(out=W2_t[:], in_=W2[:])
        b1r_t = const.tile([P, D], f32)
        nc.sync.dma_start(out=b1r_t[:], in_=b1r[:])
        b2r_t = const.tile([P, D], f32)
        nc.sync.dma_start(out=b2r_t[:], in_=b2r[:])
        from concourse.masks import make_identity
        ident = const.tile([P, P], f32)
        make_identity(nc, ident[:])
        iota_t = const.tile([P, MT, P], bf16)
        nc.sync.dma_start(out=iota_t[:], in_=iota[:])
        identbf_t = const.tile([P, P], bf16)
        nc.sync.dma_start(out=identbf_t[:], in_=identbf[:])
        dinv = const.tile([P, NT], f32)
        # local table tiles (dinv * xw, bf16) feed the self-loop identity
        # matmul; layer 2 overwrites tile t after layer 1 has consumed it.
        tabmine = const.tile([P, NT, D], bf16)

        # ---- deg / dinv ----
        with tc.tile_pool(name="degp", bufs=1) as degp:
            degw_t = degp.tile([P, NT, CAPD], f32)
            nc.sync.dma_start(out=degw_t[:],
                              in_=degw[:].rearrange("p (a b) -> p a b", a=NT))
            deg = degp.tile([P, NT], f32)
            nc.vector.tensor_reduce(out=deg[:], in_=degw_t[:],
                                    axis=mybir.AxisListType.X, op=AOP.add)
            degq = degp.tile([P, NT], f32)
            nc.vector.tensor_scalar_add(out=degq[:], in0=deg[:], scalar1=1.0)
            dsq = degp.tile([P, NT], f32)
            nc.scalar.sqrt(out=dsq[:], in_=degq[:])
            nc.vector.reciprocal(out=dinv[:], in_=dsq[:])

        # ---- layer 1 xw: tab1 = dinv * (x @ W1) ----
        with tc.tile_pool(name="xwp", bufs=3) as xwp, \
             tc.tile_pool(name="xwfull", bufs=1) as xwf, \
             tc.tile_pool(name="xwps", bufs=2, space="PSUM") as xwps:
            xT_all = xwf.tile([P, NSP], bf16)
            nc.sync.dma_start(out=xT_all[:], in_=xT[:])
            for t in range(NT):
                psx = xwps.tile([P, D], f32, tag="xw1")
                nc.tensor.matmul(out=psx[:], lhsT=xT_all[:, t * P:(t + 1) * P],
                                 rhs=W1_t[:], start=True, stop=True)
                tabt = xwp.tile([P, P], bf16, tag="tabt")
                nc.scalar.mul(tabt[:, 0:D], psx[:], dinv[:, t:t + 1])
                nc.scalar.copy(out=tabmine[:, t, :], in_=tabt[:, 0:D])
                nc.sync.dma_start(out=tab1_mine[t * P:(t + 1) * P, :],
                                  in_=tabt[:])
                for p in range(NSH - 1):
                    if t == sum(PTILES[:p + 1]) - 1:
                        nc.gpsimd.collective_compute(
                            "AllGather", AOP.bypass, replica_groups=groups,
                            ins=[tab1_mine[PB[p]:PB[p + 1], :]],
                            outs=[tab1_f[p][:]])

        nc.gpsimd.collective_compute(
            "AllGather", AOP.bypass, replica_groups=groups,
            ins=[tab1_mine[PB[3]:PB[4], :]], outs=[tab1_f[3][:]])

        # ---- edge passes ----
        def edge_pass(layer, tabs):
            has_b = has_b1 if layer == 1 else has_b2
            br_t = b1r_t if layer == 1 else b2r_t
            with tc.tile_pool(name=f"ep{layer}", bufs=3) as ep, \
                 tc.tile_pool(name=f"sel{layer}", bufs=4) as selp, \
                 tc.tile_pool(name=f"fin{layer}", bufs=3) as finp, \
                 tc.tile_pool(name=f"eps{layer}", bufs=4, space="PSUM") as epsp, \
                 tc.tile_pool(name=f"fps{layer}", bufs=2, space="PSUM") as fpsp:
                # tab2 piece p is complete once all tiles < sum(PTILES[:p+1])
                # have run their layer-1 epilogue, i.e. after group tg[p]
                tg = [(sum(PTILES[:p + 1]) + GT - 1) // GT for p in range(3)]
                for g in range(NG):
                    if layer == 1 and g in tg:
                        p = tg.index(g)
                        nc.gpsimd.collective_compute(
                            "AllGather", AOP.bypass, replica_groups=groups,
                            ins=[tab2_mine[PB[p]:PB[p + 1], :]],
                            outs=[tab2_f[p][:]])
                    cg = int(Cg[g])
                    gb = int(gbase[g])
                    ic0 = int(icb[g, 0])
                    icn = int(NIgs[g].sum() // 16)
                    idx_t = ep.tile([P, icn], i16, tag="idx")
                    nc.sync.dma_start(out=idx_t[:], in_=gidx[:, ic0:ic0 + icn])
                    dstl_t = ep.tile([P, cg], bf16, tag="dstl")
                    nc.sync.dma_start(out=dstl_t[:], in_=dstl[:, gb:gb + cg])
                    wsm_t = ep.tile([P, cg], bf16, tag="wsm")
                    nc.sync.dma_start(out=wsm_t[:], in_=wsm[:, gb:gb + cg])
                    # build this group's one-hot blocks on DVE (weight-free)
                    sel_tiles = []
                    for tloc in range(GT):
                        t = g * GT + tloc
                        M = int(ntc[t])
                        tb = int(tb_of_t[t])
                        if M == 0:
                            sel_tiles.append(None)
                            continue
                        rep = selp.tile([P, MT, P], bf16, tag="rep")
                        nc.scalar.copy(
                            out=rep[:, :M, :],
                            in_=dstl_t[:, tb:tb + M].to_broadcast([P, M, P]))
                        sel = selp.tile([P, MT, P], bf16, tag="sel")
                        nc.vector.tensor_tensor(out=sel[:, :M, :],
                                                in0=iota_t[:, :M, :],
                                                in1=rep[:, :M, :],
                                                op=AOP.is_equal)
                        sel_tiles.append(sel)
                    gbf = ep.tile([P, cg, P], bf16, tag="gbf")
                    for s in range(NSH):
                        ni = int(NIgs[g, s])
                        if ni == 0:
                            continue
                        c0 = int(cb0[g, s])
                        nchunk = ni // P
                        il0 = int(icb[g, s]) - ic0
                        nc.gpsimd.dma_gather(
                            gbf[:, c0:c0 + nchunk, :],
                            tabs[s][:],
                            idx_t[:, il0:il0 + ni // 16],
                            ni, ni, P, single_packet=False, queue_num=s)
                        wb = wsm_t[:, c0:c0 + nchunk].to_broadcast(
                            [P, nchunk, D])
                        nc.vector.tensor_tensor(
                            out=gbf[:, c0:c0 + nchunk, 0:D],
                            in0=gbf[:, c0:c0 + nchunk, 0:D], in1=wb,
                            op=AOP.mult)
                    for tloc in range(GT):
                        t = g * GT + tloc
                        M = int(ntc[t])
                        tb = int(tb_of_t[t])
                        ps_t = epsp.tile([P, D], f32, tag="eps")
                        # self-loop contribution via identity matmul
                        nc.tensor.matmul(out=ps_t[:], lhsT=identbf_t[:],
                                         rhs=tabmine[:, t, :],
                                         start=True, stop=(M == 0))
                        if M > 0:
                            sel = sel_tiles[tloc]
                            chunks = tile_chunks[t]
                            for j in range(M):
                                nc.tensor.matmul(out=ps_t[:],
                                                 lhsT=sel[:, j, :],
                                                 rhs=gbf[:, chunks[j], 0:D],
                                                 start=False,
                                                 stop=(j == M - 1))
                        h = finp.tile([P, D], f32, tag="h")
                        nc.scalar.mul(h[:], ps_t[:], dinv[:, t:t + 1])
                        if has_b:
                            hb = finp.tile([P, D], f32, tag="hb")
                            nc.vector.tensor_tensor(out=hb[:], in0=h[:],
                                                    in1=br_t[:], op=AOP.add)
                            h = hb
                        if layer == 1:
                            psT = fpsp.tile([D, P], f32, tag="tps")
                            nc.tensor.transpose(out=psT[:], in_=h[:],
                                                identity=ident[:])
                            h1T = finp.tile([D, P], f32, tag="h1T")
                            nc.scalar.copy(out=h1T[:], in_=psT[:])
                            psx2 = fpsp.tile([P, D], f32, tag="xw2")
                            nc.tensor.matmul(out=psx2[:], lhsT=h1T[:],
                                             rhs=W2_t[:], start=True, stop=True)
                            tab2t = finp.tile([P, P], bf16, tag="tab2t")
                            nc.scalar.mul(tab2t[:, 0:D], psx2[:],
                                          dinv[:, t:t + 1])
                            nc.scalar.copy(out=tabmine[:, t, :],
                                           in_=tab2t[:, 0:D])
                            nc.sync.dma_start(out=tab2_mine[t * P:(t + 1) * P, :],
                                              in_=tab2t[:]) 
                        else:
                            nc.sync.dma_start(out=h2_local[t * P:(t + 1) * P, :],
                                              in_=h[:])

        edge_pass(1, tab1_f)
        nc.gpsimd.collective_compute(
            "AllGather", AOP.bypass, replica_groups=groups,
            ins=[tab2_mine[PB[3]:PB[4], :]], outs=[tab2_f[3][:]])
        edge_pass(2, tab2_f)

        # ---- pooling head ----
        with tc.tile_pool(name="poolp", bufs=1) as pp:
            pidx_t = pp.tile([P, 32], i16)
            nc.sync.dma_start(out=pidx_t[:], in_=pidx[:])
            pbuf = pp.tile([P, 4, D], f32)
            nc.gpsimd.dma_gather(pbuf[:], h2_local[:], pidx_t[:], 512, 512, D,
                                 single_packet=False)
            pm = pp.tile([P, 2, D], f32)
            nc.vector.tensor_tensor(out=pm[:], in0=pbuf[:, 0:2, :],
                                    in1=pbuf[:, 2:4, :], op=AOP.max)
            nc.sync.dma_start(out=pool_out[:], in_=pm[:])

    nc.compile()
    return nc


LAST_RESULTS = None


def kernel(**inputs):
    global LAST_RESULTS
    from concourse.bass_utils import run_bass_kernel_spmd

    in_maps, tables, owner = _prepare(inputs)
    nc = _build(tables)
    res = run_bass_kernel_spmd(nc, in_maps, list(range(NCORES)))
    LAST_RESULTS = res
    out = np.zeros((B, D), np.float32)
    bb = np.arange(B)
    for c in range(NCORES):
        m = owner == c
        if m.any():
            po = res.results[c]["pool_out"]
            out[bb[m]] = po[bb[m] % P, bb[m] // P, :]
    return out
